# revision 10
# baseline (speedup 1.0000x reference)
"""Distributed Trainium2 kernel for a multi-head attention layer.

Problem: out = AttentionLayer(query, key, value; Wq,bq,Wk,bk,Wv,bv,Wo,bo)
  B,T,N,D,H,HD = 2,12,1024,128,8,16 ; attention runs over the N (node) axis
  independently for every (b,t) pair.

Sharding: the 24 (b,t) slabs are independent -> 3 slabs per core, no
collectives.  Each core receives its three slabs of q/k/v pre-transposed to
(D, N) layout plus replicated (pre-permuted) weights, and writes its three
output slabs in (D, N) layout; the host unshards with a pure transpose.

Per-slab device pipeline (all heads at 32-aligned partitions so the PE
32x32 sub-array tiling packs 4 small matmuls concurrently):
  1. qT/kT projections into "spread" layout (head j of group g at
     partitions 32j), biases added via K=1 rank-1 matmuls.
  2. v projection into an interleaved layout (head vals | ones cols) so the
     PV matmul simultaneously accumulates the softmax denominator.
  3. Per (group, m-chunk, head): QK^T scores (transposed orientation,
     K=16 row-tiled), exp on ACT (scale fused), PV accumulate (col-tiled).
  4. Normalization: reciprocal of denominators, PE "spread" matmul
     broadcasts 1/s across partitions, DVE multiply.
  5. Output projection with zero-padded permuted Wo; bias via rank-1 matmul.
"""

import os
import sys

import numpy as np

sys.path.insert(0, "/opt/trn_rl_repo")

import concourse.bass as bass  # noqa: E402
import concourse.tile as tile  # noqa: E402
from concourse import bacc  # noqa: E402
from concourse.tile import add_dep_helper  # noqa: E402
from concourse import mybir  # noqa: E402
from concourse._compat import with_exitstack  # noqa: E402
from concourse.bass_utils import run_bass_kernel_spmd  # noqa: E402

B, T, N, D, H, HD = 2, 12, 1024, 128, 8, 16
NCORES = 8
SLABS = (B * T) // NCORES  # 3 slabs per core
F32 = mybir.dt.float32
F32R = mybir.dt.float32r
BF16 = mybir.dt.bfloat16
SCALE = 1.0 / np.sqrt(np.float32(HD))  # 0.25
PACKW = 3080

# heads whose exp runs as a quadratic Taylor series on the DVE instead of
# the (bottleneck) ACT engine.  j-indices within each group of 4.
TAYLOR_J: tuple[int, ...] = ()


@with_exitstack
def _build_kernel(ctx, tc: "tile.TileContext", P: dict):
    nc = tc.nc

    const = ctx.enter_context(tc.tile_pool(name="const", bufs=1))
    inp = ctx.enter_context(tc.tile_pool(name="inp", bufs=2))
    qtp = ctx.enter_context(tc.tile_pool(name="qtp", bufs=2))
    vilp = ctx.enter_context(tc.tile_pool(name="vilp", bufs=2))
    expp = ctx.enter_context(tc.tile_pool(name="expp", bufs=4))
    attnp = ctx.enter_context(tc.tile_pool(name="attnp", bufs=2))
    rsp = ctx.enter_context(tc.tile_pool(name="rsp", bufs=2))
    outp = ctx.enter_context(tc.tile_pool(name="outp", bufs=2))
    pmm = ctx.enter_context(tc.tile_pool(name="pmm", bufs=3, space="PSUM"))
    pu = ctx.enter_context(tc.tile_pool(name="pu", bufs=2, space="PSUM"))

    # ---- constants: ONE packed DMA so consumers wait on one queue ----
    wpack = const.tile([D, PACKW], BF16, tag="wpack")
    wpack_dma = nc.sync.dma_start(wpack[:], P["wpack"][:])
    wqt = [wpack[:, 0:128], wpack[:, 128:256]]
    wkt = [wpack[:, 256:384], wpack[:, 384:512]]
    wot = [wpack[:, 512:640], wpack[:, 640:768]]
    hspread = wpack[:, 768:896]
    wvt_pad = wpack[:, 896:1152]
    c256 = wpack[:, 1152:1408]
    brow_q = [wpack[0:1, 1408:1536], wpack[0:1, 1536:1664]]
    brow_k = [wpack[0:1, 1664:1792], wpack[0:1, 1792:1920]]
    brow_fin = wpack[0:1, 1920:2048]
    ones_row = wpack[0:1, 2048:2560]
    ones_blk = wpack[:, 2560:3072]
    zbias = const.tile([D, 1], F32, tag="zbias")
    nc.vector.memset(zbias[:], 0.0)

    Exp = mybir.ActivationFunctionType.Exp

    def observe(producers):
        """PE nops dep'd on producer instructions so following matmuls
        (1 HW wait slot only) need not carry multiple waits."""
        nops = []
        for p in producers:
            if p is None:
                continue
            n = nc.tensor.nop(nofuse=True)
            add_dep_helper(n.ins, p.ins, reason="matmul wait split")
            nops.append(n)
        return nops

    def pin_after(mm, nops):
        for n in nops:
            add_dep_helper(mm.ins, n.ins, sync=False, reason="pin after observe")

    wpack_nop = observe([wpack_dma])
    last_mul = None

    for s in range(SLABS):
        # ---- load (already d-major) inputs ----
        xv = inp.tile([D, N], BF16, tag="xv")
        dma_v = nc.sync.dma_start(xv[:], P["xv"][s])
        xq = inp.tile([D, N], BF16, tag="xq")
        dma_q = nc.sync.dma_start(xq[:], P["xq"][s])
        xk = inp.tile([D, N], BF16, tag="xk")
        dma_k = nc.sync.dma_start(xk[:], P["xk"][s])
        in_nops = observe([dma_v, dma_q, dma_k]) + (wpack_nop if s == 0 else [])

        # ---- v projection into interleaved (vals | ones) layout ----
        # (before q/k so the later QK-matmul DVE wait also covers vil)
        vil = vilp.tile([D, 8 * 256], BF16, tag="vil")
        for mc in range(8):
            ps = pmm.tile([D, N], F32, tag="mm")
            mm = nc.tensor.matmul(ps[:, 0:256], xv[:, mc * 128 : (mc + 1) * 128],
                                  wvt_pad, start=True, stop=True)
            if mc == 0:
                pin_after(mm, in_nops)
            nc.vector.tensor_add(vil[:, mc * 256 : (mc + 1) * 256], ps[:, 0:256], c256)

        # ---- q/k projections into spread layout ----
        qt, kt = [], []
        for g in range(2):
            for (wt, brow, xin, dst) in (
                (wqt[g], brow_q[g], xq, qt),
                (wkt[g], brow_k[g], xk, kt),
            ):
                ps = pmm.tile([D, N], F32, tag="mm")
                for nh in range(2):
                    c = ps[:, nh * 512 : (nh + 1) * 512]
                    nc.tensor.matmul(c, wt, xin[:, nh * 512 : (nh + 1) * 512],
                                     start=True, stop=False)
                    nc.tensor.matmul(c, brow, ones_row,
                                     start=False, stop=True)
                t = qtp.tile([D, N], BF16, tag=f"{'q' if dst is qt else 'k'}{g}")
                nc.vector.tensor_copy(t[:], ps[:])
                dst.append(t)

        # ---- attention, two groups of four heads ----
        at = []
        for g in range(2):
            grp_nops = observe([last_mul])
            u = [pu.tile([D, 512], F32, tag="u", name=f"u{g}_{nh}")
                 for nh in range(2)]
            for mc in range(8):
                for j in range(4):
                    sc = pmm.tile([D, N], F32, tag="mm")
                    for nh in range(2):
                        mm = nc.tensor.matmul(
                            sc[:, nh * 512 : (nh + 1) * 512],
                            kt[g][32 * j : 32 * j + 16, mc * 128 : (mc + 1) * 128],
                            qt[g][32 * j : 32 * j + 16, nh * 512 : (nh + 1) * 512],
                            start=True, stop=True, tile_position=(32 * j, 0),
                        )
                        if mc == 0 and j == 0:
                            pin_after(mm, grp_nops)
                    lhs_v = vil[:, mc * 256 + g * 128 + 32 * j : mc * 256 + g * 128 + 32 * j + 32]
                    if j in TAYLOR_J:
                        # exp(x) ~ 1 + z, z = x + x^2/2, x = s/4:
                        #   y = s/32 + 1/4 ; z = s*y  (both on DVE)
                        y = expp.tile([D, N], F32, tag="ty")
                        nc.vector.tensor_scalar(y[:], sc[:], 1.0 / 32.0, 0.25,
                                                mybir.AluOpType.mult, mybir.AluOpType.add)
                        z = expp.tile([D, N], BF16, tag="tz")
                        nc.vector.tensor_tensor(z[:], sc[:], y[:], mybir.AluOpType.mult)
                        for nh in range(2):
                            nc.tensor.matmul(u[nh][32 * j : 32 * j + 32, :], lhs_v,
                                             z[:, nh * 512 : (nh + 1) * 512],
                                             start=(mc == 0), stop=False,
                                             tile_position=(0, 32 * j))
                            nc.tensor.matmul(u[nh][32 * j : 32 * j + 32, :], lhs_v,
                                             ones_blk,
                                             start=False, stop=(mc == 7),
                                             tile_position=(0, 32 * j))
                    else:
                        ex = expp.tile([D, N], BF16, tag="ex")
                        nc.scalar.activation(ex[:], sc[:], Exp, bias=zbias[:, 0:1],
                                             scale=float(SCALE))
                        for nh in range(2):
                            nc.tensor.matmul(u[nh][32 * j : 32 * j + 32, :], lhs_v,
                                             ex[:, nh * 512 : (nh + 1) * 512],
                                             start=(mc == 0), stop=(mc == 7),
                                             tile_position=(0, 32 * j))

            # ---- normalization ----
            rrec = rsp.tile([D, N], BF16, tag="rrec")
            with nc.allow_low_precision(reason="1/s fits f32r"):
                nc.vector.reciprocal(rrec[:, 0:512], u[0][:])
                nc.vector.reciprocal(rrec[:, 512:1024], u[1][:])
            rps = pmm.tile([D, N], F32, tag="mm")
            for nh in range(2):
                nc.tensor.matmul(rps[:, nh * 512 : (nh + 1) * 512], hspread,
                                 rrec[:, nh * 512 : (nh + 1) * 512],
                                 start=True, stop=True)
            rsb = rsp.tile([D, N], F32, tag="rsb")
            nc.vector.tensor_copy(rsb[:], rps[:])
            a = attnp.tile([D, N], BF16, tag=f"at{g}")
            for nh in range(2):
                last_mul = nc.vector.tensor_mul(a[:, nh * 512 : (nh + 1) * 512], u[nh][:],
                                                rsb[:, nh * 512 : (nh + 1) * 512])
            at.append(a)

        # ---- output projection ----
        fin = pmm.tile([D, N], F32, tag="mm")
        for nh in range(2):
            c = fin[:, nh * 512 : (nh + 1) * 512]
            nc.tensor.matmul(c, wot[0][:], at[0][:, nh * 512 : (nh + 1) * 512],
                             start=True, stop=False)
            nc.tensor.matmul(c, wot[1][:], at[1][:, nh * 512 : (nh + 1) * 512],
                             start=False, stop=False)
            nc.tensor.matmul(c, brow_fin[:], ones_row[:], start=False, stop=True)
        ot = outp.tile([D, N], F32, tag="ot")
        nc.vector.tensor_copy(ot[:], fin[:])
        nc.sync.dma_start(P["out"][s], ot[:])


_CACHE: dict = {}


def _get_nc():
    if "nc" in _CACHE:
        return _CACHE["nc"]
    nc = bacc.Bacc()
    P = {}
    for name, shape in (
        ("xq", (SLABS, D, N)), ("xk", (SLABS, D, N)), ("xv", (SLABS, D, N)),
        ("wpack", (D, PACKW)),
    ):
        P[name] = nc.declare_dram_parameter(name, list(shape), BF16, isOutput=False)
    P["out"] = nc.declare_dram_parameter("out", [SLABS, D, N], F32, isOutput=True)

    with tile.TileContext(nc) as tc:
        _build_kernel(tc, P)
    nc.finalize()
    _CACHE["nc"] = nc
    return nc


def _spread_w(W, off):
    """(128,128) lhsT for q/k projection: head j of this group at cols 32j."""
    A = np.zeros((D, D), np.float32)
    for j in range(4):
        A[:, 32 * j : 32 * j + 16] = W[off + 16 * j : off + 16 * j + 16, :].T
    return A


def _spread_b(b, off):
    r = np.zeros((1, D), np.float32)
    for j in range(4):
        r[0, 32 * j : 32 * j + 16] = b[off + 16 * j : off + 16 * j + 16]
    return r


def _host_consts(Wq, bq, Wk, bk, Wv, bv, Wo, bo):
    pack = np.zeros((D, PACKW), np.float32)
    pack[:, 0:128] = _spread_w(Wq, 0)
    pack[:, 128:256] = _spread_w(Wq, 64)
    pack[:, 256:384] = _spread_w(Wk, 0)
    pack[:, 384:512] = _spread_w(Wk, 64)
    wo_a = np.zeros((D, D), np.float32)
    wo_b = np.zeros((D, D), np.float32)
    for j in range(4):
        wo_a[32 * j : 32 * j + 16, :] = Wo[:, 16 * j : 16 * j + 16].T
        wo_b[32 * j : 32 * j + 16, :] = Wo[:, 64 + 16 * j : 64 + 16 * j + 16].T
    pack[:, 512:640] = wo_a
    pack[:, 640:768] = wo_b
    hs = np.zeros((D, D), np.float32)
    for p in range(D):
        hs[32 * (p // 32) + 16, p] = 1.0
    pack[:, 768:896] = hs
    wvt = np.zeros((D, 256), np.float32)
    c256 = np.zeros((D, 256), np.float32)
    for g in range(2):
        for j in range(4):
            h = 4 * g + j
            base = g * 128 + 32 * j
            wvt[:, base : base + 16] = Wv[16 * h : 16 * h + 16, :].T
            c256[:, base + 16 : base + 32] = 1.0
    pack[:, 896:1152] = wvt
    pack[:, 1152:1408] = c256
    pack[0, 1408:1536] = _spread_b(bq, 0)[0]
    pack[0, 1536:1664] = _spread_b(bq, 64)[0]
    pack[0, 1664:1792] = _spread_b(bk, 0)[0]
    pack[0, 1792:1920] = _spread_b(bk, 64)[0]
    pack[0, 1920:2048] = (Wo @ bv + bo).astype(np.float32)
    pack[0, 2048:2560] = 1.0
    pack[:, 2560:3072] = 1.0
    import ml_dtypes
    return {"wpack": pack.astype(ml_dtypes.bfloat16)}


def _host_consts_OLD(Wq, bq, Wk, bk, Wv, bv, Wo, bo):
    consts = {
        "wqt_a": _spread_w(Wq, 0), "wqt_b": _spread_w(Wq, 64),
        "wkt_a": _spread_w(Wk, 0), "wkt_b": _spread_w(Wk, 64),
        "brow_q_a": _spread_b(bq, 0), "brow_q_b": _spread_b(bq, 64),
        "brow_k_a": _spread_b(bk, 0), "brow_k_b": _spread_b(bk, 64),
    }
    # v projection weights: group g heads at cols g*128 + 32j .. +16
    wvt = np.zeros((D, 256), np.float32)
    c256 = np.zeros((D, 256), np.float32)
    for g in range(2):
        for j in range(4):
            h = 4 * g + j
            base = g * 128 + 32 * j
            wvt[:, base : base + 16] = Wv[16 * h : 16 * h + 16, :].T
            c256[:, base + 16 : base + 32] = 1.0
    consts["wvt_pad"] = wvt
    consts["c256"] = c256
    # spread matrix: R[p] = rrec[32*(p//32)+16]
    hs = np.zeros((D, D), np.float32)
    for p in range(D):
        hs[32 * (p // 32) + 16, p] = 1.0
    consts["hspread"] = hs
    # output projection, U-layout rows (junk rows zero)
    wo_a = np.zeros((D, D), np.float32)
    wo_b = np.zeros((D, D), np.float32)
    for j in range(4):
        wo_a[32 * j : 32 * j + 16, :] = Wo[:, 16 * j : 16 * j + 16].T
        wo_b[32 * j : 32 * j + 16, :] = Wo[:, 64 + 16 * j : 64 + 16 * j + 16].T
    consts["wot_a"] = wo_a
    consts["wot_b"] = wo_b
    # rows of attn sum to 1 exactly, so bv folds through Wo; bo added too.
    consts["brow_fin"] = (Wo @ bv + bo).reshape(1, D).astype(np.float32)
    return consts


def kernel(**inputs) -> np.ndarray:
    q = np.asarray(inputs["query"], np.float32)
    k = np.asarray(inputs["key"], np.float32)
    v = np.asarray(inputs["value"], np.float32)
    consts = _host_consts(
        *(np.asarray(inputs[n], np.float32)
          for n in ("Wq", "bq", "Wk", "bk", "Wv", "bv", "Wo", "bo"))
    )
    # slabs in (D, N) layout, bf16 for full-rate PE streams
    import ml_dtypes
    bf = ml_dtypes.bfloat16
    qT = np.ascontiguousarray(q.reshape(B * T, N, D).transpose(0, 2, 1)).astype(bf)
    kT = np.ascontiguousarray(k.reshape(B * T, N, D).transpose(0, 2, 1)).astype(bf)
    vT = np.ascontiguousarray(v.reshape(B * T, N, D).transpose(0, 2, 1)).astype(bf)

    nc = _get_nc()
    in_maps = []
    for c in range(NCORES):
        sl = slice(SLABS * c, SLABS * (c + 1))
        m = {"xq": qT[sl], "xk": kT[sl], "xv": vT[sl]}
        m.update(consts)
        in_maps.append(m)

    res = run_bass_kernel_spmd(nc, in_maps, core_ids=list(range(NCORES)),
                               trace=bool(int(os.environ.get("KERNEL_TRACE", "0"))))
    _CACHE["last_result"] = res
    out = np.concatenate([res.results[c]["out"] for c in range(NCORES)], axis=0)
    return np.ascontiguousarray(
        out.transpose(0, 2, 1).reshape(B, T, N, D)).astype(np.float32)


# revision 12
# speedup vs baseline: 1.1626x; 1.1626x over previous
"""Distributed Trainium2 kernel for a multi-head attention layer.

Problem: out = AttentionLayer(query, key, value; Wq,bq,Wk,bk,Wv,bv,Wo,bo)
  B,T,N,D,H,HD = 2,12,1024,128,8,16 ; attention runs over the N (node) axis
  independently for every (b,t) pair.

Sharding: the 24 (b,t) slabs are independent -> 3 slabs per core, no
collectives.  Each core receives its three slabs of q/k/v pre-transposed to
(D, N) layout (bf16) plus replicated pre-permuted weights, and writes its
three output slabs in (D, N) f32 layout; the host unshards with a transpose.

Per-slab device pipeline (heads at 32-aligned partitions):
  1. v projection into an interleaved layout (head vals | ones cols) so the
     PV matmul simultaneously accumulates the softmax denominator.
  2. qT/kT projections into "spread" layout (head j of group g at
     partitions 32j); biases folded into the PSUM->SBUF copy (tensor_scalar).
  3. Per (group, m-chunk): 4 heads' QK^T scores (transposed orientation,
     K=16), exp on ACT (scale fused, FD=1024), then the 8 PV matmuls
     emitted adjacently so the PE runs them 4-way col-group concurrent.
  4. Normalization: reciprocal_approx_fast on denominators, PE "spread"
     matmul broadcasts 1/s across partitions, DVE multiply.
  5. Output projection with zero-padded permuted Wo; bias folded into the
     output copy.
"""

import os
import sys

import numpy as np

sys.path.insert(0, "/opt/trn_rl_repo")

import concourse.bass as bass  # noqa: E402,F401
import concourse.tile as tile  # noqa: E402
from concourse import bacc  # noqa: E402
from concourse import mybir  # noqa: E402
from concourse._compat import with_exitstack  # noqa: E402
from concourse.bass_utils import run_bass_kernel_spmd  # noqa: E402

B, T, N, D, H, HD = 2, 12, 1024, 128, 8, 16
NCORES = 8
SLABS = (B * T) // NCORES  # 3 slabs per core
F32 = mybir.dt.float32
BF16 = mybir.dt.bfloat16
SCALE = 1.0 / np.sqrt(np.float32(HD))  # 0.25
PACKW = 3104


@with_exitstack
def _build_kernel(ctx, tc: "tile.TileContext", P: dict):
    nc = tc.nc

    const = ctx.enter_context(tc.tile_pool(name="const", bufs=1))
    inp = ctx.enter_context(tc.tile_pool(name="inp", bufs=2))
    qtp = ctx.enter_context(tc.tile_pool(name="qtp", bufs=2))
    vilp = ctx.enter_context(tc.tile_pool(name="vilp", bufs=2))
    expp = ctx.enter_context(tc.tile_pool(name="expp", bufs=6))
    attnp = ctx.enter_context(tc.tile_pool(name="attnp", bufs=2))
    rsp = ctx.enter_context(tc.tile_pool(name="rsp", bufs=2))
    outp = ctx.enter_context(tc.tile_pool(name="outp", bufs=2))
    pmm = ctx.enter_context(tc.tile_pool(name="pmm", bufs=3, space="PSUM"))
    pu = ctx.enter_context(tc.tile_pool(name="pu", bufs=2, space="PSUM"))

    # ---- constants: ONE packed DMA ----
    wpack = const.tile([D, PACKW], BF16, tag="wpack")
    nc.sync.dma_start(wpack[:], P["wpack"][:])
    wqt = [wpack[:, 0:128], wpack[:, 128:256]]
    wkt = [wpack[:, 256:384], wpack[:, 384:512]]
    wot = [wpack[:, 512:640], wpack[:, 640:768]]
    hspread = wpack[:, 768:896]
    wvt_pad = wpack[:, 896:1152]
    c256 = wpack[:, 1152:1408]
    # per-partition bias columns (spread layouts, f32 for tensor_scalar)
    bpack = const.tile([D, 8], F32, tag="bpack")
    nc.sync.dma_start(bpack[:], P["bpack"][:])
    bq_col = [bpack[:, 0:1], bpack[:, 1:2]]
    bk_col = [bpack[:, 2:3], bpack[:, 3:4]]
    bfin_col = bpack[:, 4:5]

    zbias = const.tile([D, 1], F32, tag="zbias")
    nc.vector.memset(zbias[:], 0.0)

    Exp = mybir.ActivationFunctionType.Exp
    ADD = mybir.AluOpType.add

    for s in range(SLABS):
        # ---- load (already d-major, bf16) inputs ----
        xv = inp.tile([D, N], BF16, tag="xv")
        nc.sync.dma_start(xv[:], P["xv"][s])
        xq = inp.tile([D, N], BF16, tag="xq")
        nc.sync.dma_start(xq[:], P["xq"][s])
        xk = inp.tile([D, N], BF16, tag="xk")
        nc.sync.dma_start(xk[:], P["xk"][s])

        # ---- v projection into interleaved (vals | ones) layout ----
        vil = vilp.tile([D, 8 * 256], BF16, tag="vil")
        for mc in range(8):
            ps = pmm.tile([D, N], F32, tag="mm")
            nc.tensor.matmul(ps[:, 0:256], xv[:, mc * 128 : (mc + 1) * 128],
                             wvt_pad, start=True, stop=True)
            nc.vector.tensor_add(vil[:, mc * 256 : (mc + 1) * 256], ps[:, 0:256], c256)

        # ---- q/k projections into spread layout; bias via tensor_scalar ----
        qt, kt = [], []
        for g in range(2):
            for (wt, bcol, xin, dst, tg) in (
                (wqt[g], bq_col[g], xq, qt, f"q{g}"),
                (wkt[g], bk_col[g], xk, kt, f"k{g}"),
            ):
                ps = pmm.tile([D, N], F32, tag="mm")
                for nh in range(2):
                    nc.tensor.matmul(ps[:, nh * 512 : (nh + 1) * 512], wt,
                                     xin[:, nh * 512 : (nh + 1) * 512],
                                     start=True, stop=True)
                t = qtp.tile([D, N], BF16, tag=tg)
                nc.vector.tensor_scalar(t[:], ps[:], bcol, None, ADD)
                dst.append(t)

        # ---- attention, two groups of four heads ----
        at = []
        for g in range(2):
            u = [pu.tile([D, 512], F32, tag="u", name=f"u{g}_{nh}")
                 for nh in range(2)]
            for mc in range(8):
                exs = []
                for j in range(4):
                    sc = pmm.tile([D, N], F32, tag="mm", name=f"sc{j}")
                    for nh in range(2):
                        nc.tensor.matmul(
                            sc[:, nh * 512 : (nh + 1) * 512],
                            kt[g][32 * j : 32 * j + 16, mc * 128 : (mc + 1) * 128],
                            qt[g][32 * j : 32 * j + 16, nh * 512 : (nh + 1) * 512],
                            start=True, stop=True, tile_position=(32 * j, 0),
                        )
                    ex = expp.tile([D, N], BF16, tag="ex", name=f"ex{j}")
                    nc.scalar.activation(ex[:], sc[:], Exp, bias=zbias[:, 0:1],
                                         scale=float(SCALE))
                    exs.append(ex)
                # 8 PV matmuls emitted adjacently -> 4-way col-group overlap
                for nh in range(2):
                    for j in range(4):
                        lo = mc * 256 + g * 128 + 32 * j
                        nc.tensor.matmul(u[nh][32 * j : 32 * j + 32, :],
                                         vil[:, lo : lo + 32],
                                         exs[j][:, nh * 512 : (nh + 1) * 512],
                                         start=(mc == 0), stop=(mc == 7),
                                         tile_position=(0, 32 * j))

            # ---- normalization ----
            rtmp = rsp.tile([D, N], F32, tag="rtmp")
            nc.vector.reciprocal_approx_fast(rtmp[:, 0:512], u[0][:])
            nc.vector.reciprocal_approx_fast(rtmp[:, 512:1024], u[1][:])
            rrec = rsp.tile([D, N], BF16, tag="rrec")
            nc.vector.tensor_copy(rrec[:], rtmp[:])
            rps = pmm.tile([D, N], F32, tag="mm")
            for nh in range(2):
                nc.tensor.matmul(rps[:, nh * 512 : (nh + 1) * 512], hspread,
                                 rrec[:, nh * 512 : (nh + 1) * 512],
                                 start=True, stop=True)
            rsb = rsp.tile([D, N], F32, tag="rsb")
            nc.vector.tensor_copy(rsb[:], rps[:])
            a = attnp.tile([D, N], BF16, tag=f"at{g}")
            for nh in range(2):
                nc.vector.tensor_mul(a[:, nh * 512 : (nh + 1) * 512], u[nh][:],
                                     rsb[:, nh * 512 : (nh + 1) * 512])
            at.append(a)

        # ---- output projection; bias folded into the output copy ----
        fin = pmm.tile([D, N], F32, tag="mm")
        for nh in range(2):
            c = fin[:, nh * 512 : (nh + 1) * 512]
            nc.tensor.matmul(c, wot[0], at[0][:, nh * 512 : (nh + 1) * 512],
                             start=True, stop=False)
            nc.tensor.matmul(c, wot[1], at[1][:, nh * 512 : (nh + 1) * 512],
                             start=False, stop=True)
        ot = outp.tile([D, N], F32, tag="ot")
        nc.vector.tensor_scalar(ot[:], fin[:], bfin_col, None, ADD)
        nc.sync.dma_start(P["out"][s], ot[:])


_CACHE: dict = {}


def _get_nc():
    if "nc" in _CACHE:
        return _CACHE["nc"]
    nc = bacc.Bacc()
    P = {}
    for name, shape in (
        ("xq", (SLABS, D, N)), ("xk", (SLABS, D, N)), ("xv", (SLABS, D, N)),
        ("wpack", (D, PACKW)),
    ):
        P[name] = nc.declare_dram_parameter(name, list(shape), BF16, isOutput=False)
    P["bpack"] = nc.declare_dram_parameter("bpack", [D, 8], F32, isOutput=False)
    P["out"] = nc.declare_dram_parameter("out", [SLABS, D, N], F32, isOutput=True)

    with tile.TileContext(nc) as tc:
        _build_kernel(tc, P)
    nc.finalize()
    _CACHE["nc"] = nc
    return nc


def _spread_w(W, off):
    """(128,128) lhsT for q/k projection: head j of this group at cols 32j."""
    A = np.zeros((D, D), np.float32)
    for j in range(4):
        A[:, 32 * j : 32 * j + 16] = W[off + 16 * j : off + 16 * j + 16, :].T
    return A


def _spread_b(b, off):
    r = np.zeros(D, np.float32)
    for j in range(4):
        r[32 * j : 32 * j + 16] = b[off + 16 * j : off + 16 * j + 16]
    return r


def _host_consts(Wq, bq, Wk, bk, Wv, bv, Wo, bo):
    pack = np.zeros((D, PACKW), np.float32)
    pack[:, 0:128] = _spread_w(Wq, 0)
    pack[:, 128:256] = _spread_w(Wq, 64)
    pack[:, 256:384] = _spread_w(Wk, 0)
    pack[:, 384:512] = _spread_w(Wk, 64)
    wo_a = np.zeros((D, D), np.float32)
    wo_b = np.zeros((D, D), np.float32)
    for j in range(4):
        wo_a[32 * j : 32 * j + 16, :] = Wo[:, 16 * j : 16 * j + 16].T
        wo_b[32 * j : 32 * j + 16, :] = Wo[:, 64 + 16 * j : 64 + 16 * j + 16].T
    pack[:, 512:640] = wo_a
    pack[:, 640:768] = wo_b
    hs = np.zeros((D, D), np.float32)
    for p in range(D):
        hs[32 * (p // 32) + 16, p] = 1.0
    pack[:, 768:896] = hs
    wvt = np.zeros((D, 256), np.float32)
    c256 = np.zeros((D, 256), np.float32)
    for g in range(2):
        for j in range(4):
            h = 4 * g + j
            base = g * 128 + 32 * j
            wvt[:, base : base + 16] = Wv[16 * h : 16 * h + 16, :].T
            c256[:, base + 16 : base + 32] = 1.0
    pack[:, 896:1152] = wvt
    pack[:, 1152:1408] = c256
    bp = np.zeros((D, 8), np.float32)
    bp[:, 0] = _spread_b(bq, 0)
    bp[:, 1] = _spread_b(bq, 64)
    bp[:, 2] = _spread_b(bk, 0)
    bp[:, 3] = _spread_b(bk, 64)
    bp[:, 4] = (Wo @ bv + bo).astype(np.float32)
    import ml_dtypes
    return {"wpack": pack.astype(ml_dtypes.bfloat16), "bpack": bp}


def kernel(**inputs) -> np.ndarray:
    q = np.asarray(inputs["query"], np.float32)
    k = np.asarray(inputs["key"], np.float32)
    v = np.asarray(inputs["value"], np.float32)
    consts = _host_consts(
        *(np.asarray(inputs[n], np.float32)
          for n in ("Wq", "bq", "Wk", "bk", "Wv", "bv", "Wo", "bo"))
    )
    # slabs in (D, N) layout, bf16 for full-rate PE streams
    import ml_dtypes
    bf = ml_dtypes.bfloat16
    qT = np.ascontiguousarray(q.reshape(B * T, N, D).transpose(0, 2, 1)).astype(bf)
    kT = np.ascontiguousarray(k.reshape(B * T, N, D).transpose(0, 2, 1)).astype(bf)
    vT = np.ascontiguousarray(v.reshape(B * T, N, D).transpose(0, 2, 1)).astype(bf)

    nc = _get_nc()
    in_maps = []
    for c in range(NCORES):
        sl = slice(SLABS * c, SLABS * (c + 1))
        m = {"xq": qT[sl], "xk": kT[sl], "xv": vT[sl]}
        m.update(consts)
        in_maps.append(m)

    res = run_bass_kernel_spmd(nc, in_maps, core_ids=list(range(NCORES)),
                               trace=bool(int(os.environ.get("KERNEL_TRACE", "0"))))
    _CACHE["last_result"] = res
    out = np.concatenate([res.results[c]["out"] for c in range(NCORES)], axis=0)
    return np.ascontiguousarray(
        out.transpose(0, 2, 1).reshape(B, T, N, D)).astype(np.float32)


# revision 13
# speedup vs baseline: 1.6999x; 1.4622x over previous
"""Distributed Trainium2 kernel for a multi-head attention layer.

Problem: out = AttentionLayer(query, key, value; Wq,bq,Wk,bk,Wv,bv,Wo,bo)
  B,T,N,D,H,HD = 2,12,1024,128,8,16 ; attention runs over the N (node) axis
  independently for every (b,t) pair.

Sharding: the 24 (b,t) slabs are independent -> 3 slabs per core, no
collectives.  Each core receives its three slabs of q/k/v pre-transposed to
(D, N) layout (bf16) plus replicated pre-permuted weights, and writes its
three output slabs in (D, N) f32 layout; the host unshards with a transpose.

Per-slab device pipeline (heads at 32-aligned partitions):
  1. v projection into an interleaved layout (head vals | ones cols) so the
     PV matmul simultaneously accumulates the softmax denominator.
  2. qT/kT projections into "spread" layout (head j of group g at
     partitions 32j); biases folded into the PSUM->SBUF copy (tensor_scalar).
  3. Per (group, m-chunk): 4 heads' QK^T scores (transposed orientation,
     K=16), exp on ACT (scale fused, FD=1024), then the 8 PV matmuls
     emitted adjacently so the PE runs them 4-way col-group concurrent.
  4. Normalization: reciprocal_approx_fast on denominators, PE "spread"
     matmul broadcasts 1/s across partitions, DVE multiply.
  5. Output projection with zero-padded permuted Wo; bias folded into the
     output copy.
"""

import os
import sys

import numpy as np

sys.path.insert(0, "/opt/trn_rl_repo")

import concourse.bass as bass  # noqa: E402,F401
import concourse.tile as tile  # noqa: E402
from concourse import bacc  # noqa: E402
from concourse import mybir  # noqa: E402
from concourse._compat import with_exitstack  # noqa: E402
from concourse.tile import add_dep_helper  # noqa: E402
from concourse.bass_utils import run_bass_kernel_spmd  # noqa: E402

B, T, N, D, H, HD = 2, 12, 1024, 128, 8, 16
NCORES = 8
SLABS = (B * T) // NCORES  # 3 slabs per core
F32 = mybir.dt.float32
BF16 = mybir.dt.bfloat16
SCALE = 1.0 / np.sqrt(np.float32(HD))  # 0.25
PACKW = 3104


@with_exitstack
def _build_kernel(ctx, tc: "tile.TileContext", P: dict):
    nc = tc.nc

    const = ctx.enter_context(tc.tile_pool(name="const", bufs=1))
    inp = ctx.enter_context(tc.tile_pool(name="inp", bufs=2))
    qtp = ctx.enter_context(tc.tile_pool(name="qtp", bufs=2))
    vilp = ctx.enter_context(tc.tile_pool(name="vilp", bufs=2))
    expp = ctx.enter_context(tc.tile_pool(name="expp", bufs=6))
    attnp = ctx.enter_context(tc.tile_pool(name="attnp", bufs=2))
    rsp = ctx.enter_context(tc.tile_pool(name="rsp", bufs=2))
    outp = ctx.enter_context(tc.tile_pool(name="outp", bufs=2))
    pmm = ctx.enter_context(tc.tile_pool(name="pmm", bufs=3, space="PSUM"))
    pu = ctx.enter_context(tc.tile_pool(name="pu", bufs=2, space="PSUM"))

    # ---- constants: ONE packed DMA ----
    wpack = const.tile([D, PACKW], BF16, tag="wpack")
    nc.sync.dma_start(wpack[:], P["wpack"][:])
    wqt = [wpack[:, 0:128], wpack[:, 128:256]]
    wkt = [wpack[:, 256:384], wpack[:, 384:512]]
    wot = [wpack[:, 512:640], wpack[:, 640:768]]
    hspread = wpack[:, 768:896]
    wvt_pad = wpack[:, 896:1152]
    c256 = wpack[:, 1152:1408]
    # per-partition bias columns (spread layouts, f32 for tensor_scalar)
    bpack = const.tile([D, 8], F32, tag="bpack")
    nc.sync.dma_start(bpack[:], P["bpack"][:])
    bq_col = [bpack[:, 0:1], bpack[:, 1:2]]
    bk_col = [bpack[:, 2:3], bpack[:, 3:4]]
    bfin_col = bpack[:, 4:5]

    zbias = const.tile([D, 1], F32, tag="zbias")
    nc.vector.memset(zbias[:], 0.0)

    Exp = mybir.ActivationFunctionType.Exp
    ADD = mybir.AluOpType.add

    for s in range(SLABS):
        # ---- load (already d-major, bf16) inputs ----
        xv = inp.tile([D, N], BF16, tag="xv")
        nc.sync.dma_start(xv[:], P["xv"][s])
        xq = inp.tile([D, N], BF16, tag="xq")
        nc.sync.dma_start(xq[:], P["xq"][s])
        xk = inp.tile([D, N], BF16, tag="xk")
        nc.sync.dma_start(xk[:], P["xk"][s])

        # ---- v projection into interleaved (vals | ones) layout ----
        vil = vilp.tile([D, 8 * 256], BF16, tag="vil")
        for mc in range(8):
            ps = pmm.tile([D, N], F32, tag="mm")
            nc.tensor.matmul(ps[:, 0:256], xv[:, mc * 128 : (mc + 1) * 128],
                             wvt_pad, start=True, stop=True)
            nc.vector.tensor_add(vil[:, mc * 256 : (mc + 1) * 256], ps[:, 0:256], c256)

        # ---- q/k projections into spread layout; bias via tensor_scalar ----
        qt, kt = [], []
        for g in range(2):
            for (wt, bcol, xin, dst, tg) in (
                (wqt[g], bq_col[g], xq, qt, f"q{g}"),
                (wkt[g], bk_col[g], xk, kt, f"k{g}"),
            ):
                ps = pmm.tile([D, N], F32, tag="mm")
                for nh in range(2):
                    nc.tensor.matmul(ps[:, nh * 512 : (nh + 1) * 512], wt,
                                     xin[:, nh * 512 : (nh + 1) * 512],
                                     start=True, stop=True)
                t = qtp.tile([D, N], BF16, tag=tg)
                nc.vector.tensor_scalar(t[:], ps[:], bcol, None, ADD)
                dst.append(t)

        # ---- attention, two groups of four heads ----
        at = []
        for g in range(2):
            u = [pu.tile([D, 512], F32, tag="u", name=f"u{g}_{nh}")
                 for nh in range(2)]
            for mc in range(8):
                exs, ex_insts = [], []
                for j in range(4):
                    sc = pmm.tile([D, N], F32, tag="mm", name=f"sc{j}")
                    for nh in range(2):
                        nc.tensor.matmul(
                            sc[:, nh * 512 : (nh + 1) * 512],
                            kt[g][32 * j : 32 * j + 16, mc * 128 : (mc + 1) * 128],
                            qt[g][32 * j : 32 * j + 16, nh * 512 : (nh + 1) * 512],
                            start=True, stop=True, tile_position=(32 * j, 0),
                        )
                    ex = expp.tile([D, N], BF16, tag="ex", name=f"ex{j}")
                    ei = nc.scalar.activation(ex[:], sc[:], Exp, bias=zbias[:, 0:1],
                                              scale=float(SCALE))
                    exs.append(ex)
                    ex_insts.append(ei)
                # 8 PV matmuls gated on the LAST exp and emitted at high
                # priority -> contiguous block -> 4-way col-group overlap
                with tc.high_priority():
                    for nh in range(2):
                        for j in range(4):
                            lo = mc * 256 + g * 128 + 32 * j
                            mm = nc.tensor.matmul(
                                u[nh][32 * j : 32 * j + 32, :],
                                vil[:, lo : lo + 32],
                                exs[j][:, nh * 512 : (nh + 1) * 512],
                                start=(mc == 0), stop=(mc == 7),
                                tile_position=(0, 32 * j))
                            add_dep_helper(mm.ins, ex_insts[3].ins,
                                           reason="PV quad grouping")

            # ---- normalization ----
            rtmp = rsp.tile([D, N], F32, tag="rtmp")
            nc.vector.reciprocal_approx_fast(rtmp[:, 0:512], u[0][:])
            nc.vector.reciprocal_approx_fast(rtmp[:, 512:1024], u[1][:])
            rrec = rsp.tile([D, N], BF16, tag="rrec")
            nc.vector.tensor_copy(rrec[:], rtmp[:])
            rps = pmm.tile([D, N], F32, tag="mm")
            for nh in range(2):
                nc.tensor.matmul(rps[:, nh * 512 : (nh + 1) * 512], hspread,
                                 rrec[:, nh * 512 : (nh + 1) * 512],
                                 start=True, stop=True)
            rsb = rsp.tile([D, N], F32, tag="rsb")
            nc.vector.tensor_copy(rsb[:], rps[:])
            a = attnp.tile([D, N], BF16, tag=f"at{g}")
            for nh in range(2):
                nc.vector.tensor_mul(a[:, nh * 512 : (nh + 1) * 512], u[nh][:],
                                     rsb[:, nh * 512 : (nh + 1) * 512])
            at.append(a)

        # ---- output projection; bias folded into the output copy ----
        fin = pmm.tile([D, N], F32, tag="mm")
        for nh in range(2):
            c = fin[:, nh * 512 : (nh + 1) * 512]
            nc.tensor.matmul(c, wot[0], at[0][:, nh * 512 : (nh + 1) * 512],
                             start=True, stop=False)
            nc.tensor.matmul(c, wot[1], at[1][:, nh * 512 : (nh + 1) * 512],
                             start=False, stop=True)
        ot = outp.tile([D, N], F32, tag="ot")
        nc.vector.tensor_scalar(ot[:], fin[:], bfin_col, None, ADD)
        nc.sync.dma_start(P["out"][s], ot[:])


_CACHE: dict = {}


def _get_nc():
    if "nc" in _CACHE:
        return _CACHE["nc"]
    nc = bacc.Bacc()
    P = {}
    for name, shape in (
        ("xq", (SLABS, D, N)), ("xk", (SLABS, D, N)), ("xv", (SLABS, D, N)),
        ("wpack", (D, PACKW)),
    ):
        P[name] = nc.declare_dram_parameter(name, list(shape), BF16, isOutput=False)
    P["bpack"] = nc.declare_dram_parameter("bpack", [D, 8], F32, isOutput=False)
    P["out"] = nc.declare_dram_parameter("out", [SLABS, D, N], F32, isOutput=True)

    with tile.TileContext(nc) as tc:
        _build_kernel(tc, P)
    nc.finalize()
    _CACHE["nc"] = nc
    return nc


def _spread_w(W, off):
    """(128,128) lhsT for q/k projection: head j of this group at cols 32j."""
    A = np.zeros((D, D), np.float32)
    for j in range(4):
        A[:, 32 * j : 32 * j + 16] = W[off + 16 * j : off + 16 * j + 16, :].T
    return A


def _spread_b(b, off):
    r = np.zeros(D, np.float32)
    for j in range(4):
        r[32 * j : 32 * j + 16] = b[off + 16 * j : off + 16 * j + 16]
    return r


def _host_consts(Wq, bq, Wk, bk, Wv, bv, Wo, bo):
    pack = np.zeros((D, PACKW), np.float32)
    pack[:, 0:128] = _spread_w(Wq, 0)
    pack[:, 128:256] = _spread_w(Wq, 64)
    pack[:, 256:384] = _spread_w(Wk, 0)
    pack[:, 384:512] = _spread_w(Wk, 64)
    wo_a = np.zeros((D, D), np.float32)
    wo_b = np.zeros((D, D), np.float32)
    for j in range(4):
        wo_a[32 * j : 32 * j + 16, :] = Wo[:, 16 * j : 16 * j + 16].T
        wo_b[32 * j : 32 * j + 16, :] = Wo[:, 64 + 16 * j : 64 + 16 * j + 16].T
    pack[:, 512:640] = wo_a
    pack[:, 640:768] = wo_b
    hs = np.zeros((D, D), np.float32)
    for p in range(D):
        hs[32 * (p // 32) + 16, p] = 1.0
    pack[:, 768:896] = hs
    wvt = np.zeros((D, 256), np.float32)
    c256 = np.zeros((D, 256), np.float32)
    for g in range(2):
        for j in range(4):
            h = 4 * g + j
            base = g * 128 + 32 * j
            wvt[:, base : base + 16] = Wv[16 * h : 16 * h + 16, :].T
            c256[:, base + 16 : base + 32] = 1.0
    pack[:, 896:1152] = wvt
    pack[:, 1152:1408] = c256
    bp = np.zeros((D, 8), np.float32)
    bp[:, 0] = _spread_b(bq, 0)
    bp[:, 1] = _spread_b(bq, 64)
    bp[:, 2] = _spread_b(bk, 0)
    bp[:, 3] = _spread_b(bk, 64)
    bp[:, 4] = (Wo @ bv + bo).astype(np.float32)
    import ml_dtypes
    return {"wpack": pack.astype(ml_dtypes.bfloat16), "bpack": bp}


def kernel(**inputs) -> np.ndarray:
    q = np.asarray(inputs["query"], np.float32)
    k = np.asarray(inputs["key"], np.float32)
    v = np.asarray(inputs["value"], np.float32)
    consts = _host_consts(
        *(np.asarray(inputs[n], np.float32)
          for n in ("Wq", "bq", "Wk", "bk", "Wv", "bv", "Wo", "bo"))
    )
    # slabs in (D, N) layout, bf16 for full-rate PE streams
    import ml_dtypes
    bf = ml_dtypes.bfloat16
    qT = np.ascontiguousarray(q.reshape(B * T, N, D).transpose(0, 2, 1)).astype(bf)
    kT = np.ascontiguousarray(k.reshape(B * T, N, D).transpose(0, 2, 1)).astype(bf)
    vT = np.ascontiguousarray(v.reshape(B * T, N, D).transpose(0, 2, 1)).astype(bf)

    nc = _get_nc()
    in_maps = []
    for c in range(NCORES):
        sl = slice(SLABS * c, SLABS * (c + 1))
        m = {"xq": qT[sl], "xk": kT[sl], "xv": vT[sl]}
        m.update(consts)
        in_maps.append(m)

    res = run_bass_kernel_spmd(nc, in_maps, core_ids=list(range(NCORES)),
                               trace=bool(int(os.environ.get("KERNEL_TRACE", "0"))))
    _CACHE["last_result"] = res
    out = np.concatenate([res.results[c]["out"] for c in range(NCORES)], axis=0)
    return np.ascontiguousarray(
        out.transpose(0, 2, 1).reshape(B, T, N, D)).astype(np.float32)
